# revision 79
# baseline (speedup 1.0000x reference)
"""BiDirectionalAttention (BiDAF-style) Trainium2 Bass kernel.

Full-input contract: kernel(**inputs) takes the complete unsharded inputs and
returns the full [32, 2048, 512] float32 output. Internally data-parallel over
batch: 8 NeuronCores x 4 batches each.

Per batch b (C=2048 context rows, Q=64 question rows, H=128):
  sim[c,q] = <ctx[c]*w_m, qst[q]> + <w_c, ctx[c]> + <w_q, qst[q]> + mask
  q2c      = softmax_q(sim) @ qst
  c2q      = softmax_c(max_q sim) @ ctx          (one H-vector per batch)
  out      = [ctx | q2c | ctx*q2c | ctx*c2q]     (ctx block assembled on host)

Numerics/layout choices (all aimed at the memory roofline):
  - 16-bit everywhere on the wire: ctx is fed twice, fp16 [H,C] for the sim
    matmuls (contraction over H needs H on partitions; fp16 keeps the logit
    error ~4e-3 which the softmax tolerates) and bf16 natural layout for the
    elementwise/c2q work. The 3H output columns are written bf16 in a p-major
    layout (each partition one dense 12KB HBM run) and un-permuted + upcast
    on the host. All per-batch loads are dense HBM blocks (b-major host prep).
  - no per-row max subtraction: exp() uses a constant -40 logit shift, which
    cancels in both softmax ratios; bf16's f32-like exponent range absorbs
    e^{sim-40} for |sim|<~85.
  - sim is computed TWICE on the PE, once per layout consumer:
      * c-major [128, 4, 65] per wave (col 64 = <w_c, ctx>, K=1 ones-matmul
        adds the question bias) -> row max + cwc logits for the c2q softmax;
        one fused Act exp with accumulate yields the c2q weights and sums.
      * q-major [64, 512] per wave (lhsT = w_m*qst) -> the softmax_q weights
        e^T DIRECTLY in the layout the q2c matmul needs; the exact f32
        question bias rides in as the Act exp's per-partition bias column.
    This removes all PE transposes and PSUM->SBUF eT copies.
  - q2c row sums come from N=1 ones-matmuls on the same e^T operand; a single
    DVE reciprocal+mul per wave normalizes q2c straight out of PSUM.
  - DMA: 4 small + 2 context loads per batch (SP queue) + 2 half-batch
    output stores (Act queue), every descriptor a dense >=768B run; the
    benchmark repeat loop uses a staggered semaphore reset (no full
    all-engine barrier between iterations).
"""

import os
from contextlib import ExitStack

import numpy as np

import concourse.bacc as bacc
import concourse.mybir as mybir
import concourse.tile as tile
import concourse.bass as bass
from concourse.bass import ts
from concourse.bass_utils import run_bass_kernel_spmd

F32 = mybir.dt.float32
F16 = mybir.dt.float16
BF16 = mybir.dt.bfloat16
AX = mybir.AxisListType
OP = mybir.AluOpType
AF = mybir.ActivationFunctionType

B, C, Q, H = 32, 2048, 64, 128
NCORES = 8
BP = B // NCORES      # batches per core
TP = 128              # c rows per tile (partition dim)
NT = C // TP          # 16 tiles per batch
WT = 4                # tiles per wave ([128, 4, 65] sim fits one PSUM bank)
NW = NT // WT
SHIFT = -40.0         # uniform logit shift: cancels in softmax ratios,
                      # keeps e^sim inside bf16/f32 range
NEGB = -30000.0       # question-mask bias, fp16-representable


def build_module(
    sim_safe=False,
    repeat=None,
    use_pool=0,
    onchip_ctxn=False,
    staggered=True,
    split3=2,
    out_eng="scalar",
    in_eng="sync",
    inbufs=3,
    cmode="sim",
):
    # sim_safe: CoreSim's matmul visitor asserts result.shape == out_view.shape
    # without flattening free dims, so the wave-wide bias matmul (3D strided
    # out) trips it. The per-tile variant is numerically identical.
    # repeat: wrap the whole workload in a hardware For_i loop (benchmarking
    # only - reruns the same data; output unchanged).
    # use_pool: run the two big elementwise output products on the Pool
    # (gpsimd) engine instead of DVE.
    nc = bacc.Bacc("TRN2", debug=False, num_devices=NCORES)

    # b-major context layouts: every per-batch load is one dense block
    ctx_t = nc.dram_tensor("ctx_t", [BP, H, C], F16, kind="ExternalInput")
    if onchip_ctxn:
        identF = nc.dram_tensor("identF", [H, H], F16, kind="ExternalInput")
    else:
        ctx_n = nc.dram_tensor("ctx_n", [BP, TP, NT, H], BF16, kind="ExternalInput")
    qstE = nc.dram_tensor("qstE", [Q, BP, H], BF16, kind="ExternalInput")
    rhsA = nc.dram_tensor("rhsA", [H, BP, Q + 1], F16, kind="ExternalInput")
    if cmode == "sim":
        biasW = nc.dram_tensor("biasW", [1, BP, WT * Q], F16, kind="ExternalInput")
    else:
        identB = nc.dram_tensor("identB", [H, H], BF16, kind="ExternalInput")
    biasC = nc.dram_tensor("biasC", [Q, BP], F32, kind="ExternalInput")
    # p-major output: each partition writes one dense 12KB run per batch;
    # the host un-permutes during assembly
    out = nc.dram_tensor("out", [BP, TP, NT, 3 * H], BF16, kind="ExternalOutput")
    out_ap = out.ap()

    with tile.TileContext(nc) as tc, ExitStack() as ctx:
        const = ctx.enter_context(tc.tile_pool(name="const", bufs=1))
        csm = ctx.enter_context(tc.tile_pool(name="csm", bufs=2))
        inp = ctx.enter_context(tc.tile_pool(name="inp", bufs=inbufs))
        etbuf = ctx.enter_context(tc.tile_pool(name="etbuf", bufs=3))
        small = ctx.enter_context(tc.tile_pool(name="small", bufs=2))
        outp = ctx.enter_context(tc.tile_pool(name="outp", bufs=2))
        ps_sim = ctx.enter_context(
            tc.tile_pool(name="ps_sim", bufs=2 if cmode == "sim" else 1, space="PSUM")
        )
        ps_simT = ctx.enter_context(tc.tile_pool(name="ps_simT", bufs=2, space="PSUM"))
        ps_q2c = ctx.enter_context(tc.tile_pool(name="ps_q2c", bufs=2, space="PSUM"))
        ps_misc = ctx.enter_context(
            tc.tile_pool(name="ps_misc", bufs=1 if onchip_ctxn else 2, space="PSUM")
        )
        if onchip_ctxn:
            ps_ctxT = ctx.enter_context(
                tc.tile_pool(name="ps_ctxT", bufs=1, space="PSUM")
            )

        ones_row = const.tile([1, H], F16)
        nc.vector.memset(ones_row, 1.0)
        ones_row_bf = const.tile([1, H], BF16)
        nc.vector.memset(ones_row_bf, 1.0)
        ones_c64 = const.tile([Q, 1], BF16)
        nc.vector.memset(ones_c64, 1.0)
        ones_cTP = const.tile([TP, 1], F32)
        nc.vector.memset(ones_cTP, 1.0)
        shift80_col = const.tile([TP, 1], F32)
        nc.vector.memset(shift80_col, 2.0 * SHIFT)
        shift40_col = const.tile([TP, 1], F32)
        nc.vector.memset(shift40_col, SHIFT)

        rep_ctx = (
            tc.For_i(0, repeat, 1, staggered_reset=staggered) if repeat else None
        )
        if rep_ctx is not None:
            rep_ctx.__enter__()

        # small per-core loads first (everything batch 0's first wave needs)
        qstE_sb = csm.tile([Q, BP, H], BF16, tag="qstE")
        nc.sync.dma_start(out=qstE_sb, in_=qstE.ap())
        rhsA_sb = csm.tile([H, BP, Q + 1], F16, tag="rhsA")
        nc.sync.dma_start(out=rhsA_sb, in_=rhsA.ap())
        if cmode == "sim":
            bias_sb = csm.tile([1, BP, WT * Q], F16, tag="bias")
            nc.sync.dma_start(out=bias_sb, in_=biasW.ap())
        else:
            identB_sb = csm.tile([H, H], BF16, tag="identB")
            nc.sync.dma_start(out=identB_sb, in_=identB.ap())
        biasC_sb = csm.tile([Q, BP], F32, tag="biasC")
        nc.sync.dma_start(out=biasC_sb, in_=biasC.ap())
        if onchip_ctxn:
            identF_sb = csm.tile([H, H], F16, tag="identF")
            nc.sync.dma_start(out=identF_sb, in_=identF.ap())

        for b in range(BP):
            # per-batch context loads, pipelined 3 deep by the inp pool
            ieng = getattr(nc, in_eng)
            ctxt_sb = inp.tile([H, C], F16, tag="ctxt")
            ieng.dma_start(out=ctxt_sb, in_=ctx_t.ap()[b])
            ctxn_sb = inp.tile([TP, NT, H], BF16, tag="ctxn")
            if not onchip_ctxn:
                ieng.dma_start(out=ctxn_sb, in_=ctx_n.ap()[b])

            stage = outp.tile([TP, NT, 3 * H], BF16, tag="stage")
            ssum = small.tile([TP, NT], F32, tag="ssum")
            exp_rm = small.tile([TP, NT], BF16, tag="exprm")
            rmcw = small.tile([TP, NT], F32, tag="rmcw")

            # -------- phase 1: sim -> e -> q2c, per wave of 4 c-tiles ------
            for w in range(NW):
                wsl = slice(w * WT, (w + 1) * WT)
                # q-major sim for this wave's 512 context rows: one matmul,
                # question bias added exactly (f32) during the exp
                simT = ps_simT.tile([Q, WT * TP], F32, tag="simT")
                nc.tensor.matmul(
                    simT,
                    lhsT=rhsA_sb[:, b, 0:Q],
                    rhs=ctxt_sb[:, w * WT * TP : (w + 1) * WT * TP],
                    start=True,
                    stop=True,
                )
                eT_sb = etbuf.tile([Q, WT * TP], BF16, tag="eTs")
                nc.scalar.activation(
                    out=eT_sb,
                    in_=simT,
                    func=AF.Exp,
                    bias=biasC_sb[:, b : b + 1],
                    scale=1.0,
                )

                if cmode == "sim":
                    # c-major sim: rowmax + cwc logits for the c2q softmax
                    sim = ps_sim.tile([TP, WT, Q + 1], F32, tag="sim")
                    for k in range(WT):
                        t = w * WT + k
                        nc.tensor.matmul(
                            sim[:, k, :],
                            lhsT=ctxt_sb[:, ts(t, TP)],
                            rhs=rhsA_sb[:, b, :],
                            start=(k == 0),
                            stop=False,
                        )
                    bias_w = bias_sb[:, b, :].rearrange("o (k q) -> o k q", k=WT)
                    if sim_safe:
                        for k in range(WT):
                            nc.tensor.matmul(
                                sim[:, k, 0:Q],
                                lhsT=ones_row,
                                rhs=bias_w[:, k, :],
                                start=False,
                                stop=(k == WT - 1),
                            )
                    else:
                        nc.tensor.matmul(
                            sim[:, :, 0:Q],
                            lhsT=ones_row,
                            rhs=bias_w,
                            start=False,
                            stop=True,
                        )
                    nc.vector.tensor_reduce(
                        out=rmcw[:, wsl], in_=sim[:, :, 0:Q], axis=AX.X, op=OP.max
                    )
                    nc.vector.tensor_add(rmcw[:, wsl], rmcw[:, wsl], sim[:, :, Q])
                else:
                    # c2q logits without the c-major sim:
                    #   exp(rm+cwc-80) = [max_q eT] * exp(cwc-40)
                    # rowmax via a Pool partition-reduce over eT (monotonic),
                    # cwc via N=1 matmuls on the already-loaded ctx weights
                    cwc_ps = ps_sim.tile([TP, WT], F32, tag="cwc")
                    for k in range(WT):
                        nc.tensor.matmul(
                            cwc_ps[:, k : k + 1],
                            lhsT=ctxt_sb[:, ts(w * WT + k, TP)],
                            rhs=rhsA_sb[:, b, Q : Q + 1],
                            start=(k == 0),
                            stop=(k == WT - 1),
                        )
                    cwce = small.tile([TP, WT], BF16, tag="cwce")
                    nc.scalar.activation(
                        out=cwce,
                        in_=cwc_ps,
                        func=AF.Exp,
                        bias=shift40_col,
                        scale=1.0,
                    )
                    rmrow = etbuf.tile([1, WT * TP], BF16, tag="rmrow")
                    nc.gpsimd.tensor_reduce(
                        out=rmrow, in_=eT_sb, axis=AX.C, op=OP.max
                    )
                    rmT_ps = ps_sim.tile([TP, 2 * WT], BF16, tag="rmT")
                    for k in range(WT):
                        nc.tensor.matmul(
                            rmT_ps[:, 2 * k : 2 * k + 1],
                            lhsT=rmrow[:, ts(k, TP)],
                            rhs=identB_sb[0:1, 0:1],
                            is_transpose=True,
                            start=(k == 0),
                            stop=(k == WT - 1),
                        )
                    rmT_b = bass.AP(
                        tensor=rmT_ps.tensor,
                        offset=rmT_ps.offset,
                        ap=[rmT_ps.ap[0], [2, WT]],
                    )
                    nc.vector.tensor_mul(exp_rm[:, wsl], rmT_b, cwce)

                if onchip_ctxn:
                    # natural-layout ctx for this wave via PE transposes
                    ctxT_ps = ps_ctxT.tile([TP, WT, H], F16, tag="ctxT")
                    for k in range(WT):
                        nc.tensor.matmul(
                            ctxT_ps[:, k, :],
                            lhsT=ctxt_sb[:, ts(w * WT + k, TP)],
                            rhs=identF_sb,
                            is_transpose=True,
                            start=(k == 0),
                            stop=(k == WT - 1),
                        )
                    if w % 2 == 0:
                        nc.scalar.copy(out=ctxn_sb[:, wsl, :], in_=ctxT_ps)
                    else:
                        nc.vector.tensor_copy(out=ctxn_sb[:, wsl, :], in_=ctxT_ps)

                # q2c numerators + row sums on the PE
                q2c_ps = ps_q2c.tile([TP, WT, H], F32, tag="q2c")
                s4_ps = ps_misc.tile([TP, WT], F32, tag="misc")
                for k in range(WT):
                    lhs = eT_sb[:, ts(k, TP)]
                    nc.tensor.matmul(
                        q2c_ps[:, k, :],
                        lhsT=lhs,
                        rhs=qstE_sb[:, b, :],
                        start=(k == 0),
                        stop=(k == WT - 1),
                    )
                    nc.tensor.matmul(
                        s4_ps[:, k : k + 1],
                        lhsT=lhs,
                        rhs=ones_c64,
                        start=(k == 0),
                        stop=(k == WT - 1),
                    )
                nc.vector.reciprocal(ssum[:, wsl], s4_ps)
                # q2c normalize straight out of PSUM: one mul per wave
                ss_b = bass.AP(
                    tensor=ssum.tensor,
                    offset=ssum[:, wsl].offset,
                    ap=[ssum.ap[0], [ssum.ap[1][0], WT], [0, H]],
                )
                nc.vector.tensor_mul(stage[:, wsl, 0:H], q2c_ps, ss_b)

            # -------- phase 2: softmax over c, c2q -------------------------
            psums = small.tile([TP, 1], F32, tag="psums")
            if cmode == "sim":
                nc.scalar.activation(
                    out=exp_rm,
                    in_=rmcw,
                    func=AF.Exp,
                    bias=shift80_col,
                    scale=1.0,
                    accum_out=psums,
                )
            else:
                nc.vector.tensor_reduce(out=psums, in_=exp_rm, axis=AX.X, op=OP.add)
            s2_ps = ps_misc.tile([1, 1], F32, tag="misc")
            nc.tensor.matmul(s2_ps, lhsT=psums, rhs=ones_cTP, start=True, stop=True)
            s2_r = small.tile([1, 1], F32, tag="s2r")
            nc.vector.reciprocal(s2_r, s2_ps)
            c2q_ps = ps_misc.tile([1, H], F32, tag="misc")
            for t in range(NT):
                nc.tensor.matmul(
                    c2q_ps,
                    lhsT=exp_rm[:, t : t + 1],
                    rhs=ctxn_sb[:, t, :],
                    start=(t == 0),
                    stop=(t == NT - 1),
                )
            c2q_sb = small.tile([1, H], BF16, tag="c2q")
            nc.vector.tensor_scalar_mul(c2q_sb, c2q_ps, s2_r)
            c2qb_ps = ps_misc.tile([H, H], F32, tag="misc")
            nc.tensor.matmul(
                c2qb_ps, lhsT=ones_row_bf, rhs=c2q_sb, start=True, stop=True
            )
            c2qb_sb = small.tile([H, H], BF16, tag="c2qb")
            nc.scalar.copy(out=c2qb_sb, in_=c2qb_ps)

            # -------- phase 3: elementwise outputs, chunked so the output
            # DMA starts before the whole batch's muls finish ---------------
            col2_eng = nc.gpsimd if use_pool >= 2 else nc.vector
            col3_eng = nc.gpsimd if use_pool >= 1 else nc.vector
            oeng = getattr(nc, out_eng)
            hn = NT // split3
            for j in range(split3):
                jsl = slice(j * hn, (j + 1) * hn)
                col2_eng.tensor_mul(
                    stage[:, jsl, H : 2 * H], stage[:, jsl, 0:H], ctxn_sb[:, jsl, :]
                )
                c2qb_b = bass.AP(
                    tensor=c2qb_sb.tensor,
                    offset=c2qb_sb.offset,
                    ap=[c2qb_sb.ap[0], [0, hn], c2qb_sb.ap[1]],
                )
                col3_eng.tensor_mul(
                    stage[:, jsl, 2 * H : 3 * H], ctxn_sb[:, jsl, :], c2qb_b
                )
                oeng.dma_start(out=out_ap[b][:, jsl, :], in_=stage[:, jsl, :])
        if rep_ctx is not None:
            rep_ctx.__exit__(None, None, None)

    nc.compile()
    return nc


_MODULE = None


def _get_module():
    global _MODULE
    if _MODULE is None:
        _MODULE = build_module()
    return _MODULE


def make_in_maps(context, question, question_mask, att_weight):
    """Host-side prep: sharding + layout/dtype transforms (no attention math)."""
    context = np.ascontiguousarray(np.asarray(context, np.float32))
    question = np.ascontiguousarray(np.asarray(question, np.float32))
    qmask = np.asarray(question_mask)
    att_weight = np.asarray(att_weight, np.float32)
    w_c, w_q, w_m = att_weight[:H], att_weight[H : 2 * H], att_weight[2 * H :]

    import ml_dtypes

    bf16 = ml_dtypes.bfloat16

    qmw_t = (question * w_m[None, None, :]).transpose(0, 2, 1)  # [B, H, Q]
    rhs_full = np.concatenate(
        [qmw_t, np.broadcast_to(w_c[None, :, None], (B, H, 1))], axis=2
    ).astype(np.float16)  # [B, H, Q+1]
    bias = (question @ w_q) + np.where(qmask, np.float32(0.0), np.float32(NEGB))
    bias4 = np.tile(bias.astype(np.float16), (1, WT))  # [B, WT*Q]
    bias_col = (bias + np.float32(SHIFT)).T.astype(np.float32)  # [Q, B]
    identf = np.eye(H, dtype=np.float16)

    ctx_t_full = context.transpose(0, 2, 1).astype(np.float16)  # [B, H, C]
    ctx_n_full = (
        context.reshape(B, NT, TP, H).transpose(0, 2, 1, 3).astype(bf16)
    )  # [B, TP, NT, H]
    qst_t = question.transpose(1, 0, 2).astype(bf16)  # [Q, B, H]
    rhs_t = rhs_full.transpose(1, 0, 2)  # [H, B, Q+1]

    in_maps = []
    for i in range(NCORES):
        sl = slice(i * BP, (i + 1) * BP)
        in_maps.append(
            {
                "ctx_t": np.ascontiguousarray(ctx_t_full[sl]),
                "ctx_n": np.ascontiguousarray(ctx_n_full[sl]),
                "qstE": np.ascontiguousarray(qst_t[:, sl]),
                "rhsA": np.ascontiguousarray(rhs_t[:, sl]),
                "biasW": np.ascontiguousarray(bias4[sl][None, :, :]),
                "biasC": np.ascontiguousarray(bias_col[:, sl]),
                "identF": identf,
                "identB": np.eye(H, dtype=bf16),
            }
        )
    return in_maps


OUT_NAMES = ["out"]


def filter_in_maps(nc, in_maps):
    """Drop host-prepared tensors the module variant doesn't declare."""
    names = set()
    for alloc in nc.m.functions[0].allocations:
        if isinstance(alloc, mybir.MemoryLocationSet) and alloc.kind == "ExternalInput":
            names.add(alloc.memorylocations[0].name)
    return [{k: v for k, v in m.items() if k in names} for m in in_maps]


def _unpermute(dev_out):
    """[BP, TP, NT, 3H] p-major device layout -> [BP, C, 3H]."""
    return (
        np.asarray(dev_out)
        .astype(np.float32)
        .transpose(0, 2, 1, 3)
        .reshape(BP, C, 3 * H)
    )


def assemble_core0(context, core_out):
    """Assemble core 0's batches only (for CoreSim checking)."""
    out = np.empty((BP, C, 4 * H), np.float32)
    out[:, :, :H] = np.asarray(context, np.float32)[:BP]
    out[:, :, H:] = _unpermute(core_out["out"])
    return out


def assemble_output(context, core_results):
    out = np.empty((B, C, 4 * H), np.float32)
    out[:, :, :H] = np.asarray(context, np.float32)
    for i, res in enumerate(core_results):
        out[i * BP : (i + 1) * BP, :, H:] = _unpermute(res["out"])
    return out


def run(inputs, trace=False, **kwargs):
    context = np.asarray(inputs["context"], np.float32)
    in_maps = make_in_maps(
        context,
        inputs["question"],
        inputs["question_mask"],
        inputs["att_weight"],
    )
    nc = _get_module()
    res = run_bass_kernel_spmd(
        nc,
        filter_in_maps(nc, in_maps),
        core_ids=list(range(NCORES)),
        trace=trace,
        **kwargs,
    )
    return assemble_output(context, res.results), res


def kernel(**inputs):
    out, _ = run(inputs, trace=False)
    return out


# revision 83
# speedup vs baseline: 1.0259x; 1.0259x over previous
"""BiDirectionalAttention (BiDAF-style) Trainium2 Bass kernel.

Full-input contract: kernel(**inputs) takes the complete unsharded inputs and
returns the full [32, 2048, 512] float32 output. Internally data-parallel over
batch: 8 NeuronCores x 4 batches each.

Per batch b (C=2048 context rows, Q=64 question rows, H=128):
  sim[c,q] = <ctx[c]*w_m, qst[q]> + <w_c, ctx[c]> + <w_q, qst[q]> + mask
  q2c      = softmax_q(sim) @ qst
  c2q      = softmax_c(max_q sim) @ ctx          (one H-vector per batch)
  out      = [ctx | q2c | ctx*q2c | ctx*c2q]     (ctx block assembled on host)

Numerics/layout choices (all aimed at the memory roofline):
  - 16-bit everywhere on the wire: ctx is fed twice, fp16 [H,C] for the sim
    matmuls (contraction over H needs H on partitions; fp16 keeps the logit
    error ~4e-3 which the softmax tolerates) and bf16 natural layout for the
    elementwise/c2q work. The 3H output columns are written bf16 in a p-major
    layout (each partition one dense 12KB HBM run) and un-permuted + upcast
    on the host. All per-batch loads are dense HBM blocks (b-major host prep).
  - no per-row max subtraction: exp() uses a constant -40 logit shift, which
    cancels in both softmax ratios; bf16's f32-like exponent range absorbs
    e^{sim-40} for |sim|<~85.
  - sim is computed TWICE on the PE, once per layout consumer:
      * c-major [128, 4, 65] per wave (col 64 = <w_c, ctx>, K=1 ones-matmul
        adds the question bias) -> row max + cwc logits for the c2q softmax;
        one fused Act exp with accumulate yields the c2q weights and sums.
      * q-major [64, 512] per wave (lhsT = w_m*qst) -> the softmax_q weights
        e^T DIRECTLY in the layout the q2c matmul needs; the exact f32
        question bias rides in as the Act exp's per-partition bias column.
    This removes all PE transposes and PSUM->SBUF eT copies.
  - q2c row sums come from N=1 ones-matmuls on the same e^T operand; a single
    DVE reciprocal+mul per wave normalizes q2c straight out of PSUM.
  - DMA: 4 small + 2 context loads per batch (SP queue) + 2 half-batch
    output stores (Act queue), every descriptor a dense >=768B run; the
    benchmark repeat loop uses a staggered semaphore reset (no full
    all-engine barrier between iterations).
"""

import os
from contextlib import ExitStack

import numpy as np

import concourse.bacc as bacc
import concourse.mybir as mybir
import concourse.tile as tile
import concourse.bass as bass
from concourse.bass import ts
from concourse.bass_utils import run_bass_kernel_spmd

F32 = mybir.dt.float32
F16 = mybir.dt.float16
BF16 = mybir.dt.bfloat16
AX = mybir.AxisListType
OP = mybir.AluOpType
AF = mybir.ActivationFunctionType

B, C, Q, H = 32, 2048, 64, 128
NCORES = 8
BP = B // NCORES      # batches per core
TP = 128              # c rows per tile (partition dim)
NT = C // TP          # 16 tiles per batch
WT = 4                # tiles per wave ([128, 4, 65] sim fits one PSUM bank)
NW = NT // WT
SHIFT = -40.0         # uniform logit shift: cancels in softmax ratios,
                      # keeps e^sim inside bf16/f32 range
NEGB = -30000.0       # question-mask bias, fp16-representable


def build_module(
    sim_safe=False,
    repeat=None,
    use_pool=0,
    onchip_ctxn=False,
    staggered=True,
    split3=2,
    out_eng="scalar",
    in_eng="sync",
    inbufs=3,
    cmode="sim",
):
    # sim_safe: CoreSim's matmul visitor asserts result.shape == out_view.shape
    # without flattening free dims, so the wave-wide bias matmul (3D strided
    # out) trips it. The per-tile variant is numerically identical.
    # repeat: wrap the whole workload in a hardware For_i loop (benchmarking
    # only - reruns the same data; output unchanged).
    # use_pool: run the two big elementwise output products on the Pool
    # (gpsimd) engine instead of DVE.
    nc = bacc.Bacc("TRN2", debug=False, num_devices=NCORES)

    # b-major context layouts: every per-batch load is one dense block
    ctx_t = nc.dram_tensor("ctx_t", [BP, H, C], F16, kind="ExternalInput")
    if onchip_ctxn:
        identF = nc.dram_tensor("identF", [H, H], F16, kind="ExternalInput")
    else:
        ctx_n = nc.dram_tensor("ctx_n", [BP, TP, NT, H], BF16, kind="ExternalInput")
    qstE = nc.dram_tensor("qstE", [Q, BP, H], BF16, kind="ExternalInput")
    # cols 0:Q = w_m*qst (q-major sim), col Q = w_c (pool mode),
    # cols Q+1:2Q+1 = w_m*qst + w_c (c-major sim: rowmax+cwc in one pass)
    rhsA = nc.dram_tensor("rhsA", [H, BP, 2 * Q + 1], F16, kind="ExternalInput")
    if cmode == "sim":
        biasW = nc.dram_tensor("biasW", [1, BP, WT * Q], F16, kind="ExternalInput")
    else:
        identB = nc.dram_tensor("identB", [H, H], BF16, kind="ExternalInput")
    biasC = nc.dram_tensor("biasC", [Q, BP], F32, kind="ExternalInput")
    # p-major output: each partition writes one dense 12KB run per batch;
    # the host un-permutes during assembly
    out = nc.dram_tensor("out", [BP, TP, NT, 3 * H], BF16, kind="ExternalOutput")
    out_ap = out.ap()

    with tile.TileContext(nc) as tc, ExitStack() as ctx:
        const = ctx.enter_context(tc.tile_pool(name="const", bufs=1))
        csm = ctx.enter_context(tc.tile_pool(name="csm", bufs=2))
        inp = ctx.enter_context(tc.tile_pool(name="inp", bufs=inbufs))
        etbuf = ctx.enter_context(tc.tile_pool(name="etbuf", bufs=3))
        small = ctx.enter_context(tc.tile_pool(name="small", bufs=2))
        outp = ctx.enter_context(tc.tile_pool(name="outp", bufs=2))
        ps_sim = ctx.enter_context(
            tc.tile_pool(name="ps_sim", bufs=2 if cmode == "sim" else 1, space="PSUM")
        )
        ps_simT = ctx.enter_context(tc.tile_pool(name="ps_simT", bufs=2, space="PSUM"))
        ps_q2c = ctx.enter_context(tc.tile_pool(name="ps_q2c", bufs=2, space="PSUM"))
        ps_misc = ctx.enter_context(
            tc.tile_pool(name="ps_misc", bufs=1 if onchip_ctxn else 2, space="PSUM")
        )
        if onchip_ctxn:
            ps_ctxT = ctx.enter_context(
                tc.tile_pool(name="ps_ctxT", bufs=1, space="PSUM")
            )

        ones_row = const.tile([1, H], F16)
        nc.vector.memset(ones_row, 1.0)
        ones_row_bf = const.tile([1, H], BF16)
        nc.vector.memset(ones_row_bf, 1.0)
        ones_c64 = const.tile([Q, 1], BF16)
        nc.vector.memset(ones_c64, 1.0)
        ones_cTP = const.tile([TP, 1], F32)
        nc.vector.memset(ones_cTP, 1.0)
        shift80_col = const.tile([TP, 1], F32)
        nc.vector.memset(shift80_col, 2.0 * SHIFT)
        shift40_col = const.tile([TP, 1], F32)
        nc.vector.memset(shift40_col, SHIFT)

        rep_ctx = (
            tc.For_i(0, repeat, 1, staggered_reset=staggered) if repeat else None
        )
        if rep_ctx is not None:
            rep_ctx.__enter__()

        # small per-core loads first (everything batch 0's first wave needs)
        qstE_sb = csm.tile([Q, BP, H], BF16, tag="qstE")
        nc.sync.dma_start(out=qstE_sb, in_=qstE.ap())
        rhsA_sb = csm.tile([H, BP, 2 * Q + 1], F16, tag="rhsA")
        nc.sync.dma_start(out=rhsA_sb, in_=rhsA.ap())
        if cmode == "sim":
            bias_sb = csm.tile([1, BP, WT * Q], F16, tag="bias")
            nc.sync.dma_start(out=bias_sb, in_=biasW.ap())
        else:
            identB_sb = csm.tile([H, H], BF16, tag="identB")
            nc.sync.dma_start(out=identB_sb, in_=identB.ap())
        biasC_sb = csm.tile([Q, BP], F32, tag="biasC")
        nc.sync.dma_start(out=biasC_sb, in_=biasC.ap())
        if onchip_ctxn:
            identF_sb = csm.tile([H, H], F16, tag="identF")
            nc.sync.dma_start(out=identF_sb, in_=identF.ap())

        for b in range(BP):
            # per-batch context loads, pipelined 3 deep by the inp pool
            ieng = getattr(nc, in_eng)
            ctxt_sb = inp.tile([H, C], F16, tag="ctxt")
            ieng.dma_start(out=ctxt_sb, in_=ctx_t.ap()[b])
            ctxn_sb = inp.tile([TP, NT, H], BF16, tag="ctxn")
            if not onchip_ctxn:
                ieng.dma_start(out=ctxn_sb, in_=ctx_n.ap()[b])

            stage = outp.tile([TP, NT, 3 * H], BF16, tag="stage")
            ssum = small.tile([TP, NT], F32, tag="ssum")
            exp_rm = small.tile([TP, NT], BF16, tag="exprm")
            rmcw = small.tile([TP, NT], F32, tag="rmcw")

            # -------- phase 1: sim -> e -> q2c, per wave of 4 c-tiles ------
            for w in range(NW):
                wsl = slice(w * WT, (w + 1) * WT)
                # q-major sim for this wave's 512 context rows: one matmul,
                # question bias added exactly (f32) during the exp
                simT = ps_simT.tile([Q, WT * TP], F32, tag="simT")
                nc.tensor.matmul(
                    simT,
                    lhsT=rhsA_sb[:, b, 0:Q],
                    rhs=ctxt_sb[:, w * WT * TP : (w + 1) * WT * TP],
                    start=True,
                    stop=True,
                )
                eT_sb = etbuf.tile([Q, WT * TP], BF16, tag="eTs")
                nc.scalar.activation(
                    out=eT_sb,
                    in_=simT,
                    func=AF.Exp,
                    bias=biasC_sb[:, b : b + 1],
                    scale=1.0,
                )

                if cmode == "sim":
                    # c-major sim with w_c pre-folded into every column:
                    # rowmax directly yields rmax+cwc
                    sim = ps_sim.tile([TP, WT, Q], F32, tag="sim")
                    for k in range(WT):
                        t = w * WT + k
                        nc.tensor.matmul(
                            sim[:, k, :],
                            lhsT=ctxt_sb[:, ts(t, TP)],
                            rhs=rhsA_sb[:, b, Q + 1 : 2 * Q + 1],
                            start=(k == 0),
                            stop=False,
                        )
                    bias_w = bias_sb[:, b, :].rearrange("o (k q) -> o k q", k=WT)
                    if sim_safe:
                        for k in range(WT):
                            nc.tensor.matmul(
                                sim[:, k, :],
                                lhsT=ones_row,
                                rhs=bias_w[:, k, :],
                                start=False,
                                stop=(k == WT - 1),
                            )
                    else:
                        nc.tensor.matmul(
                            sim,
                            lhsT=ones_row,
                            rhs=bias_w,
                            start=False,
                            stop=True,
                        )
                    nc.vector.tensor_reduce(
                        out=rmcw[:, wsl], in_=sim, axis=AX.X, op=OP.max
                    )
                else:
                    # c2q logits without the c-major sim:
                    #   exp(rm+cwc-80) = [max_q eT] * exp(cwc-40)
                    # rowmax via a Pool partition-reduce over eT (monotonic),
                    # cwc via N=1 matmuls on the already-loaded ctx weights
                    cwc_ps = ps_sim.tile([TP, WT], F32, tag="cwc")
                    for k in range(WT):
                        nc.tensor.matmul(
                            cwc_ps[:, k : k + 1],
                            lhsT=ctxt_sb[:, ts(w * WT + k, TP)],
                            rhs=rhsA_sb[:, b, Q : Q + 1],
                            start=(k == 0),
                            stop=(k == WT - 1),
                        )
                    cwce = small.tile([TP, WT], BF16, tag="cwce")
                    nc.scalar.activation(
                        out=cwce,
                        in_=cwc_ps,
                        func=AF.Exp,
                        bias=shift40_col,
                        scale=1.0,
                    )
                    rmrow = etbuf.tile([1, WT * TP], BF16, tag="rmrow")
                    nc.gpsimd.tensor_reduce(
                        out=rmrow, in_=eT_sb, axis=AX.C, op=OP.max
                    )
                    rmT_ps = ps_sim.tile([TP, 2 * WT], BF16, tag="rmT")
                    for k in range(WT):
                        nc.tensor.matmul(
                            rmT_ps[:, 2 * k : 2 * k + 1],
                            lhsT=rmrow[:, ts(k, TP)],
                            rhs=identB_sb[0:1, 0:1],
                            is_transpose=True,
                            start=(k == 0),
                            stop=(k == WT - 1),
                        )
                    rmT_b = bass.AP(
                        tensor=rmT_ps.tensor,
                        offset=rmT_ps.offset,
                        ap=[rmT_ps.ap[0], [2, WT]],
                    )
                    nc.vector.tensor_mul(exp_rm[:, wsl], rmT_b, cwce)

                if onchip_ctxn:
                    # natural-layout ctx for this wave via PE transposes
                    ctxT_ps = ps_ctxT.tile([TP, WT, H], F16, tag="ctxT")
                    for k in range(WT):
                        nc.tensor.matmul(
                            ctxT_ps[:, k, :],
                            lhsT=ctxt_sb[:, ts(w * WT + k, TP)],
                            rhs=identF_sb,
                            is_transpose=True,
                            start=(k == 0),
                            stop=(k == WT - 1),
                        )
                    if w % 2 == 0:
                        nc.scalar.copy(out=ctxn_sb[:, wsl, :], in_=ctxT_ps)
                    else:
                        nc.vector.tensor_copy(out=ctxn_sb[:, wsl, :], in_=ctxT_ps)

                # q2c numerators + row sums on the PE
                q2c_ps = ps_q2c.tile([TP, WT, H], F32, tag="q2c")
                s4_ps = ps_misc.tile([TP, WT], F32, tag="misc")
                for k in range(WT):
                    lhs = eT_sb[:, ts(k, TP)]
                    nc.tensor.matmul(
                        q2c_ps[:, k, :],
                        lhsT=lhs,
                        rhs=qstE_sb[:, b, :],
                        start=(k == 0),
                        stop=(k == WT - 1),
                    )
                    nc.tensor.matmul(
                        s4_ps[:, k : k + 1],
                        lhsT=lhs,
                        rhs=ones_c64,
                        start=(k == 0),
                        stop=(k == WT - 1),
                    )
                nc.vector.reciprocal(ssum[:, wsl], s4_ps)
                # q2c normalize straight out of PSUM: one mul per wave
                ss_b = bass.AP(
                    tensor=ssum.tensor,
                    offset=ssum[:, wsl].offset,
                    ap=[ssum.ap[0], [ssum.ap[1][0], WT], [0, H]],
                )
                nc.vector.tensor_mul(stage[:, wsl, 0:H], q2c_ps, ss_b)

            # -------- phase 2: softmax over c, c2q -------------------------
            psums = small.tile([TP, 1], F32, tag="psums")
            if cmode == "sim":
                nc.scalar.activation(
                    out=exp_rm,
                    in_=rmcw,
                    func=AF.Exp,
                    bias=shift80_col,
                    scale=1.0,
                    accum_out=psums,
                )
            else:
                nc.vector.tensor_reduce(out=psums, in_=exp_rm, axis=AX.X, op=OP.add)
            s2_ps = ps_misc.tile([1, 1], F32, tag="misc")
            nc.tensor.matmul(s2_ps, lhsT=psums, rhs=ones_cTP, start=True, stop=True)
            s2_r = small.tile([1, 1], F32, tag="s2r")
            nc.vector.reciprocal(s2_r, s2_ps)
            c2q_ps = ps_misc.tile([1, H], F32, tag="misc")
            for t in range(NT):
                nc.tensor.matmul(
                    c2q_ps,
                    lhsT=exp_rm[:, t : t + 1],
                    rhs=ctxn_sb[:, t, :],
                    start=(t == 0),
                    stop=(t == NT - 1),
                )
            c2q_sb = small.tile([1, H], BF16, tag="c2q")
            nc.vector.tensor_scalar_mul(c2q_sb, c2q_ps, s2_r)
            c2qb_ps = ps_misc.tile([H, H], F32, tag="misc")
            nc.tensor.matmul(
                c2qb_ps, lhsT=ones_row_bf, rhs=c2q_sb, start=True, stop=True
            )
            c2qb_sb = small.tile([H, H], BF16, tag="c2qb")
            nc.scalar.copy(out=c2qb_sb, in_=c2qb_ps)

            # -------- phase 3: elementwise outputs, chunked so the output
            # DMA starts before the whole batch's muls finish ---------------
            col2_eng = nc.gpsimd if use_pool >= 2 else nc.vector
            col3_eng = nc.gpsimd if use_pool >= 1 else nc.vector
            oeng = getattr(nc, out_eng)
            hn = NT // split3
            for j in range(split3):
                jsl = slice(j * hn, (j + 1) * hn)
                col2_eng.tensor_mul(
                    stage[:, jsl, H : 2 * H], stage[:, jsl, 0:H], ctxn_sb[:, jsl, :]
                )
                c2qb_b = bass.AP(
                    tensor=c2qb_sb.tensor,
                    offset=c2qb_sb.offset,
                    ap=[c2qb_sb.ap[0], [0, hn], c2qb_sb.ap[1]],
                )
                col3_eng.tensor_mul(
                    stage[:, jsl, 2 * H : 3 * H], ctxn_sb[:, jsl, :], c2qb_b
                )
                oeng.dma_start(out=out_ap[b][:, jsl, :], in_=stage[:, jsl, :])
        if rep_ctx is not None:
            rep_ctx.__exit__(None, None, None)

    nc.compile()
    return nc


_MODULE = None


def _get_module():
    global _MODULE
    if _MODULE is None:
        _MODULE = build_module()
    return _MODULE


def make_in_maps(context, question, question_mask, att_weight):
    """Host-side prep: sharding + layout/dtype transforms (no attention math)."""
    context = np.ascontiguousarray(np.asarray(context, np.float32))
    question = np.ascontiguousarray(np.asarray(question, np.float32))
    qmask = np.asarray(question_mask)
    att_weight = np.asarray(att_weight, np.float32)
    w_c, w_q, w_m = att_weight[:H], att_weight[H : 2 * H], att_weight[2 * H :]

    import ml_dtypes

    bf16 = ml_dtypes.bfloat16

    qmw_t = (question * w_m[None, None, :]).transpose(0, 2, 1)  # [B, H, Q]
    wc_b = np.broadcast_to(w_c[None, :, None], (B, H, 1))
    rhs_full = np.concatenate(
        [qmw_t, wc_b, qmw_t + wc_b], axis=2
    ).astype(np.float16)  # [B, H, 2Q+1]
    bias = (question @ w_q) + np.where(qmask, np.float32(0.0), np.float32(NEGB))
    bias4 = np.tile(bias.astype(np.float16), (1, WT))  # [B, WT*Q]
    bias_col = (bias + np.float32(SHIFT)).T.astype(np.float32)  # [Q, B]
    identf = np.eye(H, dtype=np.float16)

    ctx_t_full = context.transpose(0, 2, 1).astype(np.float16)  # [B, H, C]
    ctx_n_full = (
        context.reshape(B, NT, TP, H).transpose(0, 2, 1, 3).astype(bf16)
    )  # [B, TP, NT, H]
    qst_t = question.transpose(1, 0, 2).astype(bf16)  # [Q, B, H]
    rhs_t = rhs_full.transpose(1, 0, 2)  # [H, B, Q+1]

    in_maps = []
    for i in range(NCORES):
        sl = slice(i * BP, (i + 1) * BP)
        in_maps.append(
            {
                "ctx_t": np.ascontiguousarray(ctx_t_full[sl]),
                "ctx_n": np.ascontiguousarray(ctx_n_full[sl]),
                "qstE": np.ascontiguousarray(qst_t[:, sl]),
                "rhsA": np.ascontiguousarray(rhs_t[:, sl]),
                "biasW": np.ascontiguousarray(bias4[sl][None, :, :]),
                "biasC": np.ascontiguousarray(bias_col[:, sl]),
                "identF": identf,
                "identB": np.eye(H, dtype=bf16),
            }
        )
    return in_maps


OUT_NAMES = ["out"]


def filter_in_maps(nc, in_maps):
    """Drop host-prepared tensors the module variant doesn't declare."""
    names = set()
    for alloc in nc.m.functions[0].allocations:
        if isinstance(alloc, mybir.MemoryLocationSet) and alloc.kind == "ExternalInput":
            names.add(alloc.memorylocations[0].name)
    return [{k: v for k, v in m.items() if k in names} for m in in_maps]


def _unpermute(dev_out):
    """[BP, TP, NT, 3H] p-major device layout -> [BP, C, 3H]."""
    return (
        np.asarray(dev_out)
        .astype(np.float32)
        .transpose(0, 2, 1, 3)
        .reshape(BP, C, 3 * H)
    )


def assemble_core0(context, core_out):
    """Assemble core 0's batches only (for CoreSim checking)."""
    out = np.empty((BP, C, 4 * H), np.float32)
    out[:, :, :H] = np.asarray(context, np.float32)[:BP]
    out[:, :, H:] = _unpermute(core_out["out"])
    return out


def assemble_output(context, core_results):
    out = np.empty((B, C, 4 * H), np.float32)
    out[:, :, :H] = np.asarray(context, np.float32)
    for i, res in enumerate(core_results):
        out[i * BP : (i + 1) * BP, :, H:] = _unpermute(res["out"])
    return out


def run(inputs, trace=False, **kwargs):
    context = np.asarray(inputs["context"], np.float32)
    in_maps = make_in_maps(
        context,
        inputs["question"],
        inputs["question_mask"],
        inputs["att_weight"],
    )
    nc = _get_module()
    res = run_bass_kernel_spmd(
        nc,
        filter_in_maps(nc, in_maps),
        core_ids=list(range(NCORES)),
        trace=trace,
        **kwargs,
    )
    return assemble_output(context, res.results), res


def kernel(**inputs):
    out, _ = run(inputs, trace=False)
    return out
